# revision 14
# baseline (speedup 1.0000x reference)
"""Conv1d kernel for Trainium2 (Bass/Tile), SPMD over 8 NeuronCores.

Problem (hardcoded): input [32, 128, 4096] f32, weight [256, 128, 9] f32,
bias [256] f32, stride=1, padding=4 -> output [32, 256, 4096] f32.

Strategy:
  - Data-parallel over batch: 4 batches per core x 8 cores.
  - Conv as 9 PSUM-accumulated matmuls per 512-wide output tile:
      out[co, w] = sum_k sum_ci W[co, ci, k] * xpad[ci, w + k]
    with C_in=128 as the matmul contraction (partition) dim.
  - x and w are cast to float16 on the HOST: fp16 matmul streams at
    1 cycle/row (4x faster than fp32), enables fast-weight-load, and
    halves the input DMA bytes. PSUM accumulation stays fp32; output
    rel err ~5e-4 vs the fp32 reference.

Timing model (from ntff traces): exec_time is measured from the first
framework const-memset (~6.1us after t0) to the end of the runtime's
fixed ~8us semaphore-clear postamble.  The matmul stream runs at the
fp16 roofline (216ns/MM at N=512 warm), so the winnable time is head
latency (DMA queue spin-up ~1.3us + transfers before MM#0), the HAM
cold-clock penalty, and the drain chain after the last matmul.

Head: only the sync (SP, q1) and scalar (ACT, q10) HW-DGE queues are
fast (~160 GB/s each; gpsimd DMA is software-DGE at ~20 GB/s - never
use it for bulk).  sync carries xboot then the x chunks; scalar carries
the weights - split per tap-group so the first matmul's weights (cc0,
k0:3, 98KB) land ~0.5us before the full blob would - then the output
stores.  A ~22-matmul N=128 warmup stream on a memset tile keeps the PE
busy from ~7.1us so the HAM clock-gate opens (2.4 GHz) before/shortly
after the real stream starts.

Tail: the last 512-tile is computed as two N=256 accumulation groups in
separate PSUM banks so the final bias+store chain is half as long, and
the two final stores go out on different queues (scalar, sync).
"""

import sys

if "/opt/trn_rl_repo" not in sys.path:
    sys.path.insert(0, "/opt/trn_rl_repo")

import numpy as np

import concourse.bacc as bacc
import concourse.bass as bass
import concourse.mybir as mybir
import concourse.tile as tile
from concourse.bass_utils import run_bass_kernel_spmd

F32 = mybir.dt.float32
F16 = mybir.dt.float16

N_CORES = 8
B, C_IN, W = 32, 128, 4096
C_OUT, KS = 256, 9
PAD = 4
B_LOC = B // N_CORES          # batches per core
WP = W + 2 * PAD              # padded width
CC = C_OUT // 128             # out-channel chunks of 128
WT = 512                      # output tile width (one PSUM bank of f32)
OW = 2048                     # output staging tile width
XC = 1024                     # x chunk stride (output cols covered per chunk)
XCW = XC + 2 * PAD            # x chunk width incl. halo
N_XC = W // XC                # x chunks per batch (4)
XB = WT + 2 * PAD             # bootstrap width (520)
N_WARM = 26                   # N=128 warmup matmuls (~113ns each cold)

LAST_RESULT = None            # set by kernel(); test.py reads exec_time_ns


def build_nc():
    nc = bacc.Bacc("TRN2", target_bir_lowering=False)

    # x supplied as [B_LOC, N_XC, C_IN, XCW]: pre-chunked on host with halos
    x = nc.declare_dram_parameter("x", [B_LOC, N_XC, C_IN, XCW], F16, isOutput=False)
    # first 520 cols of batch 0 again: a small bootstrap load so the first
    # matmul group starts before chunk 0 fully lands
    xboot = nc.declare_dram_parameter("xboot", [C_IN, XB], F16, isOutput=False)
    w = nc.declare_dram_parameter("w", [C_IN, CC, KS, 128], F16, isOutput=False)
    bvec = nc.declare_dram_parameter("b", [128, CC], F32, isOutput=False)
    out = nc.declare_dram_parameter("out", [B_LOC, C_OUT, W], F32, isOutput=True)

    with tile.TileContext(nc) as tc:
        with (
            tc.tile_pool(name="const", bufs=1) as cpool,
            tc.tile_pool(name="xc", bufs=2) as xpool,  # 2 slots per chunk tag
            tc.tile_pool(name="oout", bufs=4) as opool,
            tc.tile_pool(name="ps", bufs=6, space=bass.MemorySpace.PSUM) as pspool,
            tc.tile_pool(name="wps", bufs=1, space=bass.MemorySpace.PSUM) as wpspool,
        ):
            # PE warmup: HAM un-throttles (1.2 -> 2.4 GHz) after ~3.4us of
            # sustained PE activity.  Small N=128 matmuls on a memset tile
            # fill the window between the entry barrier and the bootstrap
            # DMAs landing, so the real stream runs mostly warm.
            dummy = cpool.tile([C_IN, 136], F16)
            nc.gpsimd.memset(dummy[:], 0.0)
            wps = wpspool.tile([128, WT], F32)
            for _ in range(N_WARM):
                nc.tensor.matmul(
                    wps[:, :128], dummy[:, :128], dummy[:, 8:136], start=True, stop=True
                )

            # Bootstrap loads: sync=xboot then x chunks; scalar=weights
            # per-cc (splitting finer makes the in-flight jobs interleave at
            # descriptor level on q10 and lands the later taps LATER) then
            # bias then out stores.
            xb_sb = cpool.tile([C_IN, XB], F16)
            nc.sync.dma_start(xb_sb[:], xboot[:])
            w_sb = cpool.tile([C_IN, CC, KS, 128], F16)
            nc.scalar.dma_start(w_sb[:, 0], w[:, 0])
            nc.scalar.dma_start(w_sb[:, 1], w[:, 1])
            b_sb = cpool.tile([128, CC], F32)
            nc.scalar.dma_start(b_sb[:], bvec[:])

            for bi in range(B_LOC):
                x_sb = []
                for c in range(N_XC):
                    xt = xpool.tile([C_IN, XCW], F16, tag=f"xc{c}")
                    nc.sync.dma_start(xt[:], x[bi, c])
                    x_sb.append(xt)
                for cc in range(CC):
                    for oh in range(W // OW):
                        last_slab = (
                            bi == B_LOC - 1 and cc == CC - 1 and oh == W // OW - 1
                        )
                        o_sb = opool.tile([128, OW], F32)
                        for wi in range(OW // WT):
                            wt = oh * (OW // WT) + wi
                            xc = (wt * WT) // XC          # chunk index
                            xo = wt * WT - xc * XC        # offset within chunk
                            if bi == 0 and cc == 0 and wt == 0:
                                src, so = xb_sb, 0        # bootstrap tile
                            else:
                                src, so = x_sb[xc], xo
                            ob = o_sb[:, wi * WT : (wi + 1) * WT]
                            if last_slab and wi == OW // WT - 1:
                                # final 512-tile as two N=256 groups in two
                                # PSUM banks: the drain of the first half
                                # overlaps the matmuls of the second, so the
                                # post-stream chain is bias(256)+store(256)
                                # instead of bias(512)+store(512).
                                for h in range(2):
                                    ps = pspool.tile([128, WT], F32)
                                    ho = h * 256
                                    for k in range(KS):
                                        nc.tensor.matmul(
                                            ps[:, :256],
                                            w_sb[:, cc, k, :],
                                            src[:, so + ho + k : so + ho + k + 256],
                                            start=(k == 0),
                                            stop=(k == KS - 1),
                                        )
                                    nc.vector.tensor_scalar_add(
                                        o_sb[:, wi * WT + ho : wi * WT + ho + 256],
                                        ps[:, :256],
                                        b_sb[:, cc : cc + 1],
                                    )
                                    dst = out[
                                        bi,
                                        cc * 128 : (cc + 1) * 128,
                                        oh * OW + wi * WT + ho :
                                        oh * OW + wi * WT + ho + 256,
                                    ]
                                    osrc = o_sb[:, wi * WT + ho : wi * WT + ho + 256]
                                    # both on scalar: its queue is hot (it
                                    # stored every slab); an idle queue costs
                                    # ~1.5us of re-spin-up on the final store
                                    nc.scalar.dma_start(dst, osrc)
                                continue
                            ps = pspool.tile([128, WT], F32)
                            for k in range(KS):
                                nc.tensor.matmul(
                                    ps[:],
                                    w_sb[:, cc, k, :],
                                    src[:, so + k : so + k + WT],
                                    start=(k == 0),
                                    stop=(k == KS - 1),
                                )
                            nc.vector.tensor_scalar_add(ob, ps[:], b_sb[:, cc : cc + 1])
                            if last_slab:
                                # last slab: store per-512 so the final queue
                                # drain after the last matmul stays short
                                nc.scalar.dma_start(
                                    out[
                                        bi,
                                        cc * 128 : (cc + 1) * 128,
                                        oh * OW + wi * WT : oh * OW + (wi + 1) * WT,
                                    ],
                                    ob,
                                )
                        if not last_slab:
                            nc.scalar.dma_start(
                                out[bi, cc * 128 : (cc + 1) * 128, oh * OW : (oh + 1) * OW],
                                o_sb[:],
                            )

    nc.finalize()
    return nc


def _prep_inputs(input, weight, bias):
    """Host-side shard prep. Returns per-core input maps."""
    input = np.ascontiguousarray(input, dtype=np.float32)
    weight = np.ascontiguousarray(weight, dtype=np.float32)
    bias = np.ascontiguousarray(bias, dtype=np.float32)

    xpad = np.zeros((B, C_IN, WP), dtype=np.float16)
    xpad[:, :, PAD : PAD + W] = input.astype(np.float16)

    # chunk with halo: [B, N_XC, C_IN, XCW]
    xch = np.empty((B, N_XC, C_IN, XCW), dtype=np.float16)
    for c in range(N_XC):
        xch[:, c] = xpad[:, :, c * XC : c * XC + XCW]
    xch = np.ascontiguousarray(xch)

    # [C_out, C_in, K] -> [ci, cc, k, co_in_chunk]
    wt = np.ascontiguousarray(
        weight.astype(np.float16).reshape(CC, 128, C_IN, KS).transpose(2, 0, 3, 1)
    )
    bt = np.ascontiguousarray(bias.reshape(CC, 128).T)  # [128, CC]

    in_maps = []
    for c in range(N_CORES):
        xc_core = np.ascontiguousarray(xch[c * B_LOC : (c + 1) * B_LOC])
        in_maps.append(
            {
                "x": xc_core,
                "xboot": np.ascontiguousarray(xc_core[0, 0, :, :XB]),
                "w": wt,
                "b": bt,
            }
        )
    return in_maps


def kernel(input, weight, bias, _trace=False):
    global LAST_RESULT
    in_maps = _prep_inputs(input, weight, bias)
    nc = build_nc()
    res = run_bass_kernel_spmd(nc, in_maps, list(range(N_CORES)), trace=_trace)
    LAST_RESULT = res
    out = np.concatenate([r["out"] for r in res.results], axis=0)
    return out


# revision 16
# speedup vs baseline: 1.1947x; 1.1947x over previous
"""Conv1d kernel for Trainium2 (Bass/Tile), SPMD over 8 NeuronCores.

Problem (hardcoded): input [32, 128, 4096] f32, weight [256, 128, 9] f32,
bias [256] f32, stride=1, padding=4 -> output [32, 256, 4096] f32.

Strategy:
  - Data-parallel over batch: 4 batches per core x 8 cores.
  - Conv as 9 PSUM-accumulated matmuls per 512-wide output tile:
      out[co, w] = sum_k sum_ci W[co, ci, k] * xpad[ci, w + k]
    with C_in=128 as the matmul contraction (partition) dim.
  - x and w are cast to float16 on the HOST: fp16 matmul streams at
    1 cycle/row (4x faster than fp32), enables fast-weight-load, and
    halves the input DMA bytes. PSUM accumulation stays fp32; output
    rel err ~5e-4 vs the fp32 reference.

Timing model (from ntff traces): exec_time is measured from the first
framework const-memset (~6.1us after t0) to the end of the runtime's
fixed ~8us semaphore-clear postamble.  The matmul stream runs at the
fp16 roofline (216ns/MM at N=512 warm), so the winnable time is head
latency (DMA queue spin-up ~1.3us + transfers before MM#0), the HAM
cold-clock penalty, and the drain chain after the last matmul.

Head: only the sync (SP, q1) and scalar (ACT, q10) HW-DGE queues are
fast (~160 GB/s each; gpsimd DMA is software-DGE at ~20 GB/s - never
use it for bulk).  sync carries xboot then the x chunks; scalar carries
the weights (one DMA per cc - finer splits interleave at descriptor
level on the queue and land the later taps LATER) then bias and the
output stores.  A 26-matmul N=128 warmup stream on a memset tile keeps
the PE busy from ~7.0us so the HAM clock-gate opens (2.4 GHz) around
when the real stream starts (the flip lands on a free-running 3.4us
window boundary, so +-1us run-to-run variance here is irreducible).

Tail: the last 512-tile is computed as N=384 + N=128 accumulation
groups in separate PSUM banks: the drain of the 384-half overlaps the
matmuls of the 128-half, so the post-stream chain is only
bias(128)+store(128) on the already-hot scalar queue.

Run-to-run variance: the chip sometimes executes a whole run with the
PE at 2.0 GHz instead of 2.4 (P0 power state; gap 259ns vs 216ns per
matmul) - that is environmental, not kernel-dependent.
"""

import sys

if "/opt/trn_rl_repo" not in sys.path:
    sys.path.insert(0, "/opt/trn_rl_repo")

import numpy as np

import concourse.bacc as bacc
import concourse.bass as bass
import concourse.mybir as mybir
import concourse.tile as tile
from concourse.bass_utils import run_bass_kernel_spmd

F32 = mybir.dt.float32
F16 = mybir.dt.float16

N_CORES = 8
B, C_IN, W = 32, 128, 4096
C_OUT, KS = 256, 9
PAD = 4
B_LOC = B // N_CORES          # batches per core
WP = W + 2 * PAD              # padded width
CC = C_OUT // 128             # out-channel chunks of 128
WT = 512                      # output tile width (one PSUM bank of f32)
OW = 2048                     # output staging tile width
XC = 1024                     # x chunk stride (output cols covered per chunk)
XCW = XC + 2 * PAD            # x chunk width incl. halo
N_XC = W // XC                # x chunks per batch (4)
XB = WT + 2 * PAD             # bootstrap width (520)
N_WARM = 26                   # N=128 warmup matmuls (~113ns each cold)

LAST_RESULT = None            # set by kernel(); test.py reads exec_time_ns


def build_nc():
    nc = bacc.Bacc("TRN2", target_bir_lowering=False)

    # x supplied as [B_LOC, N_XC, C_IN, XCW]: pre-chunked on host with halos
    x = nc.declare_dram_parameter("x", [B_LOC, N_XC, C_IN, XCW], F16, isOutput=False)
    # first 520 cols of batch 0 again: a small bootstrap load so the first
    # matmul group starts before chunk 0 fully lands
    xboot = nc.declare_dram_parameter("xboot", [C_IN, XB], F16, isOutput=False)
    w = nc.declare_dram_parameter("w", [C_IN, CC, KS, 128], F16, isOutput=False)
    bvec = nc.declare_dram_parameter("b", [128, CC], F32, isOutput=False)
    out = nc.declare_dram_parameter("out", [B_LOC, C_OUT, W], F32, isOutput=True)

    with tile.TileContext(nc) as tc:
        with (
            tc.tile_pool(name="const", bufs=1) as cpool,
            tc.tile_pool(name="xc", bufs=2) as xpool,  # 2 slots per chunk tag
            tc.tile_pool(name="oout", bufs=4) as opool,
            tc.tile_pool(name="ps", bufs=6, space=bass.MemorySpace.PSUM) as pspool,
            tc.tile_pool(name="wps", bufs=1, space=bass.MemorySpace.PSUM) as wpspool,
        ):
            # PE warmup: HAM un-throttles (1.2 -> 2.4 GHz) after ~3.4us of
            # sustained PE activity.  Small N=128 matmuls on a memset tile
            # fill the window between the entry barrier and the bootstrap
            # DMAs landing, so the real stream runs mostly warm.
            dummy = cpool.tile([C_IN, 136], F16)
            nc.gpsimd.memset(dummy[:], 0.0)
            wps = wpspool.tile([128, WT], F32)
            for _ in range(N_WARM):
                nc.tensor.matmul(
                    wps[:, :128], dummy[:, :128], dummy[:, 8:136], start=True, stop=True
                )

            # Bootstrap loads: sync=xboot then x chunks; scalar=weights
            # per-cc (splitting finer makes the in-flight jobs interleave at
            # descriptor level on q10 and lands the later taps LATER) then
            # bias then out stores.
            xb_sb = cpool.tile([C_IN, XB], F16)
            nc.sync.dma_start(xb_sb[:], xboot[:])
            w_sb = cpool.tile([C_IN, CC, KS, 128], F16)
            nc.scalar.dma_start(w_sb[:, 0], w[:, 0])
            nc.scalar.dma_start(w_sb[:, 1], w[:, 1])
            b_sb = cpool.tile([128, CC], F32)
            nc.scalar.dma_start(b_sb[:], bvec[:])

            for bi in range(B_LOC):
                x_sb = []
                for c in range(N_XC):
                    xt = xpool.tile([C_IN, XCW], F16, tag=f"xc{c}")
                    nc.sync.dma_start(xt[:], x[bi, c])
                    x_sb.append(xt)
                for cc in range(CC):
                    for oh in range(W // OW):
                        last_slab = (
                            bi == B_LOC - 1 and cc == CC - 1 and oh == W // OW - 1
                        )
                        o_sb = opool.tile([128, OW], F32)
                        for wi in range(OW // WT):
                            wt = oh * (OW // WT) + wi
                            xc = (wt * WT) // XC          # chunk index
                            xo = wt * WT - xc * XC        # offset within chunk
                            if bi == 0 and cc == 0 and wt == 0:
                                src, so = xb_sb, 0        # bootstrap tile
                            else:
                                src, so = x_sb[xc], xo
                            ob = o_sb[:, wi * WT : (wi + 1) * WT]
                            if last_slab and wi == OW // WT - 1:
                                # final 512-tile as N=384 + N=128 groups in
                                # two PSUM banks: the drain of the 384-half
                                # overlaps the matmuls of the 128-half, so
                                # the post-stream chain is bias(128)+
                                # store(128) instead of bias(512)+store(512).
                                for ho, hw in ((0, 384), (384, 128)):
                                    ps = pspool.tile([128, WT], F32)
                                    for k in range(KS):
                                        nc.tensor.matmul(
                                            ps[:, :hw],
                                            w_sb[:, cc, k, :],
                                            src[:, so + ho + k : so + ho + k + hw],
                                            start=(k == 0),
                                            stop=(k == KS - 1),
                                        )
                                    nc.vector.tensor_scalar_add(
                                        o_sb[:, wi * WT + ho : wi * WT + ho + hw],
                                        ps[:, :hw],
                                        b_sb[:, cc : cc + 1],
                                    )
                                    dst = out[
                                        bi,
                                        cc * 128 : (cc + 1) * 128,
                                        oh * OW + wi * WT + ho :
                                        oh * OW + wi * WT + ho + hw,
                                    ]
                                    osrc = o_sb[:, wi * WT + ho : wi * WT + ho + hw]
                                    # both on scalar: its queue is hot (it
                                    # stored every slab); an idle queue costs
                                    # ~1.5us of re-spin-up on the final store
                                    nc.scalar.dma_start(dst, osrc)
                                continue
                            ps = pspool.tile([128, WT], F32)
                            for k in range(KS):
                                nc.tensor.matmul(
                                    ps[:],
                                    w_sb[:, cc, k, :],
                                    src[:, so + k : so + k + WT],
                                    start=(k == 0),
                                    stop=(k == KS - 1),
                                )
                            nc.vector.tensor_scalar_add(ob, ps[:], b_sb[:, cc : cc + 1])
                            if last_slab:
                                # last slab: store per-512 so the final queue
                                # drain after the last matmul stays short
                                nc.scalar.dma_start(
                                    out[
                                        bi,
                                        cc * 128 : (cc + 1) * 128,
                                        oh * OW + wi * WT : oh * OW + (wi + 1) * WT,
                                    ],
                                    ob,
                                )
                        if not last_slab:
                            nc.scalar.dma_start(
                                out[bi, cc * 128 : (cc + 1) * 128, oh * OW : (oh + 1) * OW],
                                o_sb[:],
                            )

    nc.finalize()
    return nc


def _prep_inputs(input, weight, bias):
    """Host-side shard prep. Returns per-core input maps."""
    input = np.ascontiguousarray(input, dtype=np.float32)
    weight = np.ascontiguousarray(weight, dtype=np.float32)
    bias = np.ascontiguousarray(bias, dtype=np.float32)

    xpad = np.zeros((B, C_IN, WP), dtype=np.float16)
    xpad[:, :, PAD : PAD + W] = input.astype(np.float16)

    # chunk with halo: [B, N_XC, C_IN, XCW]
    xch = np.empty((B, N_XC, C_IN, XCW), dtype=np.float16)
    for c in range(N_XC):
        xch[:, c] = xpad[:, :, c * XC : c * XC + XCW]
    xch = np.ascontiguousarray(xch)

    # [C_out, C_in, K] -> [ci, cc, k, co_in_chunk]
    wt = np.ascontiguousarray(
        weight.astype(np.float16).reshape(CC, 128, C_IN, KS).transpose(2, 0, 3, 1)
    )
    bt = np.ascontiguousarray(bias.reshape(CC, 128).T)  # [128, CC]

    in_maps = []
    for c in range(N_CORES):
        xc_core = np.ascontiguousarray(xch[c * B_LOC : (c + 1) * B_LOC])
        in_maps.append(
            {
                "x": xc_core,
                "xboot": np.ascontiguousarray(xc_core[0, 0, :, :XB]),
                "w": wt,
                "b": bt,
            }
        )
    return in_maps


def kernel(input, weight, bias, _trace=False):
    global LAST_RESULT
    in_maps = _prep_inputs(input, weight, bias)
    nc = build_nc()
    res = run_bass_kernel_spmd(nc, in_maps, list(range(N_CORES)), trace=_trace)
    LAST_RESULT = res
    out = np.concatenate([r["out"] for r in res.results], axis=0)
    return out
